# revision 19
# baseline (speedup 1.0000x reference)
# Bass/Tile kernel for nn_Decoder: 4-layer dense transformer, B=2 L=2048 D=1024 H=16 V=32000.
# Sharding: token-parallel over 8 cores (core c owns the 512-token quarter c%4 of batch c//4),
# per-layer K and V AllGathers (bf16) within batch groups [[0-3],[4-7]], token-sharded
# full-vocab logits (no final collective). Weights/KV/activations bf16, residual fp32.
# Causality via per-core 0/1 mask input data (program identical across cores - SPMD).
import contextlib
import numpy as np
import concourse.bass as bass
import concourse.mybir as mybir
import concourse.tile as tile
from concourse import bacc

P = 128
D = 1024
H = 16
DH = 64
FF = 2048
L = 2048
B = 2
V = 32000
NCORE = 8
T = 512            # own tokens per core
KO = D // P        # 8
FKO = FF // P      # 16
NKT = (4 * T) // P  # 16 key tiles (full batch)
VN = 500           # vocab N-tile width
NVT = V // VN      # 64
VROW = H * (DH + 1)  # 1040: v row with ones column per head
EPS = 1e-6
f32 = mybir.dt.float32
bf16 = mybir.dt.bfloat16
AF = mybir.ActivationFunctionType
ALU = mybir.AluOpType


def build(n_layers=4, do_logits=True, has_bias=False, nonzero_bv=False,
          nonzero_blog=False, want_xout=False):
    nc = bacc.Bacc(None, target_bir_lowering=False)

    x0T = nc.dram_tensor("x0T", [D, T], f32, kind="ExternalInput")
    wq = nc.dram_tensor("wq", [n_layers, D, D], bf16, kind="ExternalInput")
    wk = nc.dram_tensor("wk", [n_layers, D, D], bf16, kind="ExternalInput")
    wv = nc.dram_tensor("wv", [n_layers, D, D], bf16, kind="ExternalInput")
    wo = nc.dram_tensor("wo", [n_layers, D, D], bf16, kind="ExternalInput")
    wproj = nc.dram_tensor("wproj", [n_layers, D, FF], bf16, kind="ExternalInput")
    wup = nc.dram_tensor("wup", [n_layers, D, FF], bf16, kind="ExternalInput")
    wdown = nc.dram_tensor("wdown", [n_layers, FF, D], bf16, kind="ExternalInput")
    consts = nc.dram_tensor("consts", [P, 2 * P], bf16, kind="ExternalInput")
    # consts[:, 0:P] = ones; consts[:, P:P+2] used as bpat (col j<64 -> row0=1,
    # j>=64 -> row1=1) -- see host_inputs.
    maskbig = nc.dram_tensor("maskbig", [NKT, P, T], bf16, kind="ExternalInput")
    # bias columns: [bq 0:8 | bk 8:16 | bo 16:24 | bproj 24:40 | bup 40:56 | bdown 56:64]
    ball = (nc.dram_tensor("ball", [n_layers, P, 64], f32, kind="ExternalInput")
            if has_bias else None)
    bvmat = (nc.dram_tensor("bvmat", [n_layers, P, D], bf16, kind="ExternalInput")
             if nonzero_bv else None)
    e0_d = (nc.dram_tensor("e0_d", [P, P], bf16, kind="ExternalInput")
            if (nonzero_bv or nonzero_blog) else None)
    if do_logits:
        wlog = nc.dram_tensor("wlog", [D, V], bf16, kind="ExternalInput")
        blogmat = (nc.dram_tensor("blogmat", [P, V], bf16, kind="ExternalInput")
                   if nonzero_blog else None)
        logits = nc.dram_tensor("logits", [T, V], f32, kind="ExternalOutput")
    if want_xout:
        xout = nc.dram_tensor("xout", [P, KO, T], f32, kind="ExternalOutput")

    with tile.TileContext(nc) as tc, contextlib.ExitStack() as ctx:
        cn = ctx.enter_context(tc.tile_pool(name="cn", bufs=1))
        pb = ctx.enter_context(tc.tile_pool(name="pb", bufs=1))
        evn = ctx.enter_context(tc.tile_pool(name="evn", bufs=2))   # norm/small temps
        eva = ctx.enter_context(tc.tile_pool(name="eva", bufs=3))   # evict stream temps
        evf = ctx.enter_context(tc.tile_pool(name="evf", bufs=2))   # f32 evict temps
        exp_ = ctx.enter_context(tc.tile_pool(name="exp", bufs=6))  # exp'd score tiles
        aca = ctx.enter_context(tc.tile_pool(name="aca", bufs=2))   # attn acc evacs
        ps = ctx.enter_context(tc.tile_pool(name="ps", bufs=3, space="PSUM"))
        pa = ctx.enter_context(tc.tile_pool(name="pa", bufs=3, space="PSUM"))
        av = ctx.enter_context(tc.tile_pool(name="av", bufs=2, space="PSUM"))
        dr = ctx.enter_context(tc.tile_pool(name="dram", bufs=2, space="DRAM"))

        # ---- constants ----
        cst = cn.tile([P, 2 * P], bf16, tag="cst")
        nc.sync.dma_start(cst[:], consts[:])
        ones = cst[:, 0:P]           # [P, P] all-ones (bf16)
        bpat = cst[:, P:P + P]       # rows 0/1: head-pair broadcast pattern
        sc_rms = cn.tile([P, 1], f32, tag="sc_rms")
        nc.any.memset(sc_rms[:], 1.0 / D)
        eps_t = cn.tile([P, 1], f32, tag="eps")
        nc.any.memset(eps_t[:], EPS)
        sc_att = cn.tile([P, 1], f32, tag="sc_att")
        nc.any.memset(sc_att[:], 0.125)
        e0 = None
        if e0_d is not None:
            e0 = cn.tile([P, P], bf16, tag="e0")
            nc.sync.dma_start(e0[:], e0_d[:])
        maskS = []
        for kt in range(NKT):
            m = cn.tile([P, T], bf16, tag=f"mask{kt}")
            nc.sync.dma_start(m[:], maskbig[kt])
            maskS.append(m)

        # ---- persistent activations ----
        xT = pb.tile([P, KO, T], f32, tag="xT")
        QT = pb.tile([P, KO, T], bf16, tag="QT")

        # ---- Phase 0: load xT (host already did embed+pe+transpose) ----
        nc.sync.dma_start(xT[:], x0T.rearrange("(ko p) t -> p ko t", p=P))

        def rmsnorm(nm):
            normT = pb.tile([P, KO, T], bf16, tag="nta", name=nm)
            ssq = ps.tile([P, T], f32, tag="mm512")
            for ko in range(KO):
                x2 = evn.tile([P, T], bf16, tag="x2")
                nc.vector.tensor_tensor(x2[:], xT[:, ko], xT[:, ko], ALU.mult)
                nc.tensor.matmul(ssq[:], ones[:], x2[:], start=(ko == 0),
                                 stop=(ko == KO - 1))
            srt = evn.tile([P, T], f32, tag="srt")
            nc.scalar.activation(srt[:], ssq[:], AF.Sqrt, bias=eps_t[:], scale=sc_rms[:])
            inv = evn.tile([P, T], f32, tag="inv")
            nc.vector.reciprocal(inv[:], srt[:])
            nc.vector.tensor_tensor(
                normT[:], xT[:], inv[:, None, :].to_broadcast([P, KO, T]), ALU.mult)
            return normT

        with tc.tile_pool(name="wp", bufs=3) as wp, \
             tc.tile_pool(name="wdp", bufs=2) as wdp, \
             tc.tile_pool(name="kvp", bufs=1) as kvp, \
             tc.tile_pool(name="acp", bufs=1) as acp:

            def proj_T(w_, i, src, bias_col, consume, n_mt=KO, n_ko=KO):
                w3 = w_[i].rearrange("(ko p) m -> p ko m", p=P)
                for m in range(n_mt):
                    wt = wp.tile([P, n_ko, P], bf16, tag="wlhs")
                    nc.sync.dma_start(wt[:], w3[:, :, m * P:(m + 1) * P])
                    pt = ps.tile([P, T], f32, tag="mm512")
                    for ko in range(n_ko):
                        nc.tensor.matmul(pt[:], wt[:, ko], src[:, ko], start=(ko == 0),
                                         stop=(ko == n_ko - 1))
                    consume(m, pt, bias_col[:, m:m + 1] if bias_col is not None else None)

            for li in range(n_layers):
                bias_t = None
                if has_bias:
                    bias_t = evn.tile([P, 64], f32, tag="bias")
                    nc.sync.dma_start(bias_t[:], ball[li])

                normT = rmsnorm(f"norm1_{li}")

                # AG1 carries K + V-heads-0-7; AG2 carries V-heads-8-15, so the
                # first half of attention never waits on the second collective.
                HVROW = (H // 2) * (DH + 1)  # 520
                ag1_in = dr.tile([D * T + T * HVROW], bf16, tag="ag1_in")
                ag1_out = dr.tile([4, D * T + T * HVROW], bf16, tag="ag1_out")
                ag2_in = dr.tile([T * HVROW], bf16, tag="ag2_in")
                ag2_out = dr.tile([4, T * HVROW], bf16, tag="ag2_out")
                kT_view = ag1_in[0:D * T].rearrange("(d t) -> d t", t=T)
                vv1 = ag1_in[D * T:].rearrange("(t h d) -> t h d", h=H // 2, d=DH + 1)
                vv2 = ag2_in.rearrange("(t h d) -> t h d", h=H // 2, d=DH + 1)

                def k_consume(m, pt, bcol, kT_view=kT_view):
                    kt_sb = eva.tile([P, T], bf16, tag="eva")
                    if bcol is not None:
                        nc.scalar.activation(kt_sb[:], pt[:], AF.Identity, bias=bcol[:])
                    else:
                        nc.vector.tensor_copy(kt_sb[:], pt[:])
                    nc.sync.dma_start(kT_view[m * P:(m + 1) * P, :], kt_sb[:])
                proj_T(wk, li, normT, bias_t[:, 8:16] if has_bias else None, k_consume)

                # ---- V projection (natural [T, D] layout, weight as moving op) ----
                wv3 = wv[li].rearrange("(ko p) m -> ko p m", p=P)
                for nh in range(D // T):
                    vv = vv1 if nh == 0 else vv2
                    for mp in range(2):  # token-tile pairs share the weight stream
                        pts = [ps.tile([P, T], f32, tag="mm512", name=f"vpt{i}") for i in range(2)]
                        for ko in range(KO):
                            wt = wp.tile([P, T], bf16, tag="wrhs")
                            nc.sync.dma_start(wt[:], wv3[ko, :, nh * T:(nh + 1) * T])
                            for mi in range(2):
                                mt = 2 * mp + mi
                                last = (ko == KO - 1) and bvmat is None
                                nc.tensor.matmul(
                                    pts[mi][:], normT[:, ko, mt * P:(mt + 1) * P],
                                    wt[:], start=(ko == 0), stop=last)
                        for mi in range(2):
                            mt = 2 * mp + mi
                            if bvmat is not None:
                                bvt = wp.tile([P, T], bf16, tag="wrhs")
                                nc.sync.dma_start(bvt[:],
                                                  bvmat[li][:, nh * T:(nh + 1) * T])
                                nc.tensor.matmul(pts[mi][:], e0[:, :P], bvt[:],
                                                 start=False, stop=True)
                            v_sb = eva.tile([P, T], bf16, tag="eva")
                            nc.vector.tensor_copy(v_sb[:], pts[mi][:])
                            nc.sync.dma_start(
                                vv[mt * P:(mt + 1) * P, :, 0:DH],
                                v_sb.rearrange("p (h d) -> p h d", d=DH))
                    # ones column for the denominator rows of this nh's heads
                    on = eva.tile([P, KO], bf16, tag="evon")
                    nc.vector.tensor_copy(on[:], ones[:, 0:KO])
                    for tt in range(T // P):
                        nc.sync.dma_start(
                            vv[tt * P:(tt + 1) * P, :, DH:DH + 1]
                            .rearrange("p h d -> p (h d)"),
                            on[:])
                    if nh == 0:
                        nc.gpsimd.collective_compute(
                            "AllGather", ALU.bypass, ins=[ag1_in[:]],
                            outs=[ag1_out[:]],
                            replica_groups=[[0, 1, 2, 3], [4, 5, 6, 7]])
                    else:
                        nc.gpsimd.collective_compute(
                            "AllGather", ALU.bypass, ins=[ag2_in[:]],
                            outs=[ag2_out[:]],
                            replica_groups=[[0, 1, 2, 3], [4, 5, 6, 7]])

                # ---- attention ----
                # AG1-sourced tiles first on the DMA queue (K + V heads 0-7),
                # AG2-sourced (V heads 8-15) after - no head-of-line blocking.
                # Issued before the Q projection so the cache DMAs aren't
                # queued behind Q's weight loads.
                ktt, vt1, vt2 = [], [], []
                for kt in range(NKT):
                    g, off = kt // 4, (kt % 4) * P
                    kth = kvp.tile([P, KO, P], bf16, tag=f"kth{kt}", name=f"kth{kt}")
                    nc.sync.dma_start(
                        kth[:],
                        ag1_out[g, 0:D * T].rearrange("(ko p t) -> p ko t", p=P, t=T)
                        [:, :, off:off + P])
                    ktt.append(kth)
                for kt in range(NKT):
                    g, off = kt // 4, (kt % 4) * P
                    vth = kvp.tile([P, H // 2, DH + 1], bf16, tag=f"vth{kt}",
                                   name=f"vth{kt}")
                    nc.sync.dma_start(
                        vth[:],
                        ag1_out[g, D * T:].rearrange("(t h d) -> t h d", h=H // 2,
                                                     d=DH + 1)[off:off + P])
                    vt1.append(vth)
                for kt in range(NKT):
                    g, off = kt // 4, (kt % 4) * P
                    vth = kvp.tile([P, H // 2, DH + 1], bf16, tag=f"vth2_{kt}",
                                   name=f"vth2_{kt}")
                    nc.sync.dma_start(
                        vth[:],
                        ag2_out[g].rearrange("(t h d) -> t h d", h=H // 2,
                                             d=DH + 1)[off:off + P])
                    vt2.append(vth)

                def q_consume(m, pt, bcol):
                    if bcol is not None:
                        nc.scalar.activation(QT[:, m, :], pt[:], AF.Identity, bias=bcol[:])
                    else:
                        nc.vector.tensor_copy(QT[:, m, :], pt[:])
                proj_T(wq, li, normT, bias_t[:, 0:8] if has_bias else None, q_consume)

                aoT = pb.tile([P, KO, T], bf16, tag="nta", name=f"aoT_{li}")
                DEPTH = 2  # sw-pipeline: AV for kt lags its score by DEPTH tiles
                for hko in range(KO):
                    avp = [av.tile([DH + 1, T], f32, tag="avacc", name=f"avp{i}") for i in range(2)]
                    exts = {}
                    for kt in range(NKT + DEPTH):
                        if kt < NKT:
                            for hi in range(2):
                                pbase = DH * hi
                                sp = pa.tile([P, T], f32, tag="att")
                                nc.tensor.matmul(sp[:],
                                                 ktt[kt][pbase:pbase + DH, hko, :],
                                                 QT[pbase:pbase + DH, hko, :],
                                                 start=True, stop=True)
                                ext = exp_.tile([P, T], bf16, tag="exs")
                                nc.scalar.activation(ext[:], sp[:], AF.Exp,
                                                     scale=sc_att[:])
                                nc.vector.tensor_tensor(ext[:], ext[:], maskS[kt][:],
                                                        ALU.mult)
                                exts[(kt, hi)] = ext
                        if kt >= DEPTH:
                            k2 = kt - DEPTH
                            for hi in range(2):
                                h = 2 * hko + hi
                                vsl = (vt1[k2][:, h, :] if h < H // 2
                                       else vt2[k2][:, h - H // 2, :])
                                nc.tensor.matmul(avp[hi][:], vsl, exts.pop((k2, hi)),
                                                 start=(k2 == 0),
                                                 stop=(k2 == NKT - 1))
                    # softmax denominators + normalization for this head pair
                    for hi in range(2):
                        ac = aca.tile([DH + 1, T], bf16, tag=f"ac{hi}", name=f"ac{hi}")
                        nc.vector.tensor_copy(ac[:], avp[hi][:])
                        invd = evn.tile([DH + 1, T], f32, tag="invd")
                        nc.vector.reciprocal(invd[DH:DH + 1, :], avp[hi][DH:DH + 1, :])
                        invdr = evn.tile([DH + 1, T], bf16, tag="invdr")
                        nc.vector.tensor_copy(invdr[DH:DH + 1, :], invd[DH:DH + 1, :])
                        bcp = pa.tile([DH, T], f32, tag="att", name="bcp")
                        nc.tensor.matmul(bcp[:], ones[DH:DH + 1, 0:DH],
                                         invdr[DH:DH + 1, :], start=True, stop=True)
                        invb = eva.tile([DH, T], bf16, tag="invb")
                        nc.vector.tensor_copy(invb[:], bcp[:])
                        pbase = DH * hi
                        nc.vector.tensor_tensor(aoT[pbase:pbase + DH, hko, :],
                                                ac[0:DH, :], invb[:], ALU.mult)

                # ---- O projection + residual ----
                def o_consume(m, pt, bcol):
                    ot = evf.tile([P, T], f32, tag="evf")
                    if bcol is not None:
                        nc.scalar.activation(ot[:], pt[:], AF.Identity, bias=bcol[:])
                    else:
                        nc.vector.tensor_copy(ot[:], pt[:])
                    nc.vector.tensor_tensor(xT[:, m, :], xT[:, m, :], ot[:], ALU.add)
                proj_T(wo, li, aoT, bias_t[:, 16:24] if has_bias else None, o_consume)

                # ---- FFN ----
                normT = rmsnorm(f"norm2_{li}")
                hts = []
                wp3 = wproj[li].rearrange("(ko p) m -> p ko m", p=P)
                wu3 = wup[li].rearrange("(ko p) m -> p ko m", p=P)
                for m in range(FKO):
                    wtp = wp.tile([P, KO, P], bf16, tag="wlhs")
                    nc.sync.dma_start(wtp[:], wp3[:, :, m * P:(m + 1) * P])
                    ptp = ps.tile([P, T], f32, tag="mm512")
                    for ko in range(KO):
                        nc.tensor.matmul(ptp[:], wtp[:, ko], normT[:, ko], start=(ko == 0),
                                         stop=(ko == KO - 1))
                    wtu = wp.tile([P, KO, P], bf16, tag="wlhs")
                    nc.sync.dma_start(wtu[:], wu3[:, :, m * P:(m + 1) * P])
                    ptu = ps.tile([P, T], f32, tag="mm512")
                    for ko in range(KO):
                        nc.tensor.matmul(ptu[:], wtu[:, ko], normT[:, ko], start=(ko == 0),
                                         stop=(ko == KO - 1))
                    usb = evn.tile([P, T], f32, tag="uev")
                    if has_bias:
                        nc.scalar.activation(usb[:], ptu[:], AF.Identity,
                                             bias=bias_t[:, 40 + m:41 + m])
                        psb = evn.tile([P, T], f32, tag="pev")
                        nc.scalar.activation(psb[:], ptp[:], AF.Identity,
                                             bias=bias_t[:, 24 + m:25 + m])
                        gsb = evn.tile([P, T], bf16, tag="gev")
                        nc.vector.tensor_tensor(gsb[:], psb[:], usb[:], ALU.mult)
                    else:
                        nc.scalar.activation(usb[:], ptu[:], AF.Copy)
                        gsb = evn.tile([P, T], bf16, tag="gev")
                        nc.vector.tensor_tensor(gsb[:], ptp[:], usb[:], ALU.mult)
                    sgt = evn.tile([P, T], bf16, tag="sgt")
                    nc.scalar.activation(sgt[:], gsb[:], AF.Sigmoid)
                    ht = acp.tile([P, T], bf16, tag=f"acc{m}", name=f"ht{m}")
                    nc.vector.tensor_tensor(ht[:], gsb[:], sgt[:], ALU.mult)
                    hts.append(ht)
                wd3 = wdown[li].rearrange("(ko p) m -> p ko m", p=P)
                for m in range(KO):
                    wt = wdp.tile([P, FKO, P], bf16, tag="wdn")
                    nc.sync.dma_start(wt[:], wd3[:, :, m * P:(m + 1) * P])
                    pt = ps.tile([P, T], f32, tag="mm512")
                    for ko in range(FKO):
                        nc.tensor.matmul(pt[:], wt[:, ko], hts[ko][:], start=(ko == 0),
                                         stop=(ko == FKO - 1))
                    dt_ = evf.tile([P, T], f32, tag="evf")
                    if has_bias:
                        nc.scalar.activation(dt_[:], pt[:], AF.Identity,
                                             bias=bias_t[:, 56 + m:57 + m])
                    else:
                        nc.vector.tensor_copy(dt_[:], pt[:])
                    nc.vector.tensor_tensor(xT[:, m, :], xT[:, m, :], dt_[:], ALU.add)

        if want_xout:
            nc.sync.dma_start(xout[:], xT[:])

        if do_logits:
            xb = pb.tile([P, KO, T], bf16, tag="QT")
            for ko in range(KO):  # per-ko: each unblocks as layer-3 FFN finishes
                nc.vector.tensor_copy(xb[:, ko], xT[:, ko])
            wl3 = wlog.rearrange("(ko p) m -> p ko m", p=P)
            with tc.tile_pool(name="wlp", bufs=3) as wlp, \
                 tc.tile_pool(name="lev", bufs=3) as lev:
                for nt in range(NVT):
                    wt = wlp.tile([P, KO, VN], bf16, tag="wlog")
                    nc.sync.dma_start(wt[:], wl3[:, :, nt * VN:(nt + 1) * VN])
                    blt = None
                    if blogmat is not None:
                        blt = wlp.tile([P, VN], bf16, tag="wlogb")
                        nc.sync.dma_start(blt[:], blogmat[:, nt * VN:(nt + 1) * VN])
                    for mt in range(T // P):
                        pt = ps.tile([P, VN], f32, tag="mm512")
                        for ko in range(KO):
                            last = (ko == KO - 1) and blogmat is None
                            nc.tensor.matmul(pt[:], xb[:, ko, mt * P:(mt + 1) * P],
                                             wt[:, ko], start=(ko == 0), stop=last)
                        if blt is not None:
                            nc.tensor.matmul(pt[:], e0[:, :P], blt[:],
                                             start=False, stop=True)
                        lo = lev.tile([P, VN], f32, tag="evl")
                        nc.vector.tensor_copy(lo[:], pt[:])
                        nc.sync.dma_start(
                            logits[mt * P:(mt + 1) * P, nt * VN:(nt + 1) * VN], lo[:])

    nc.compile()
    return nc


def host_inputs(inp, n_layers=4, do_logits=True):
    """Build per-core in_maps from the full model inputs dict (numpy)."""
    import math
    import ml_dtypes
    bf = ml_dtypes.bfloat16
    g = {k: np.asarray(v) for k, v in inp.items()}
    ids = g["input_ids"].astype(np.int64)
    embed = g["embed"].astype(np.float32)
    pos = np.arange(L, dtype=np.float32)[:, None]
    div = np.exp(np.arange(0, D, 2, dtype=np.float32) * (-math.log(10000.0) / D))
    ang = pos * div
    pe = np.zeros((L, D), dtype=np.float32)
    pe[:, 0::2] = np.sin(ang)
    pe[:, 1::2] = np.cos(ang)

    gam = g["gammas"].astype(np.float32)
    wq_ = np.ascontiguousarray(gam[0:2 * n_layers:2, :, None] * g["Wq"][:n_layers]).astype(bf)
    wk_ = np.ascontiguousarray(gam[0:2 * n_layers:2, :, None] * g["Wk"][:n_layers]).astype(bf)
    wv_ = np.ascontiguousarray(gam[0:2 * n_layers:2, :, None] * g["Wv"][:n_layers]).astype(bf)
    wo_ = np.ascontiguousarray(g["Wo"][:n_layers]).astype(bf)
    wp_ = np.ascontiguousarray(gam[1:2 * n_layers:2, :, None] * g["Wproj"][:n_layers]).astype(bf)
    wu_ = np.ascontiguousarray(gam[1:2 * n_layers:2, :, None] * g["Wup"][:n_layers]).astype(bf)
    wd_ = np.ascontiguousarray(g["Wdown"][:n_layers]).astype(bf)

    has_bias = bool(
        np.any(g["bq"][:n_layers]) or np.any(g["bk"][:n_layers])
        or np.any(g["bo"][:n_layers]) or np.any(g["bproj"][:n_layers])
        or np.any(g["bup"][:n_layers]) or np.any(g["bdown"][:n_layers]))
    nonzero_bv = bool(np.any(g["bv"][:n_layers] != 0))
    nonzero_blog = do_logits and bool(np.any(g["blogits"] != 0))

    ball = None
    if has_bias:
        ball = np.zeros((n_layers, P, 64), np.float32)
        for i in range(n_layers):
            ball[i, :, 0:8] = g["bq"][i].reshape(8, P).T
            ball[i, :, 8:16] = g["bk"][i].reshape(8, P).T
            ball[i, :, 16:24] = g["bo"][i].reshape(8, P).T
            ball[i, :, 24:40] = g["bproj"][i].reshape(16, P).T
            ball[i, :, 40:56] = g["bup"][i].reshape(16, P).T
            ball[i, :, 56:64] = g["bdown"][i].reshape(8, P).T
    bvmat = None
    if nonzero_bv:
        bvmat = np.zeros((n_layers, P, D), np.float32)
        bvmat[:, 0, :] = g["bv"][:n_layers]
        bvmat = bvmat.astype(bf)
    e0 = np.zeros((P, P), np.float32)
    e0[0, :] = 1.0

    consts = np.zeros((P, 2 * P), np.float32)
    consts[:, 0:P] = 1.0
    consts[0, P:P + DH] = 1.0        # bpat row 0 -> out partitions 0:64
    consts[1, P + DH:P + P] = 1.0    # bpat row 1 -> out partitions 64:128
    consts = consts.astype(bf)

    wlog_bf = None
    if do_logits:
        wlog_bf = np.ascontiguousarray(g["Wlogits"]).astype(bf)
        blogmat = None
        if nonzero_blog:
            blogmat = np.zeros((P, V), np.float32)
            blogmat[0, :] = g["blogits"]
            blogmat = blogmat.astype(bf)

    in_maps = []
    for c in range(NCORE):
        b, q = c // 4, c % 4
        tok = ids[b, q * T:(q + 1) * T]
        x0 = embed[tok] + pe[q * T:(q + 1) * T, :]
        i_idx = np.arange(P)[:, None]
        j_idx = np.arange(T)[None, :]
        maskbig = np.zeros((NKT, P, T), np.float32)
        for kt in range(NKT):
            maskbig[kt] = ((kt * P + i_idx) <= (q * T + j_idx)).astype(np.float32)
        m = {
            "x0T": np.ascontiguousarray(x0.T, np.float32),
            "wq": wq_, "wk": wk_, "wv": wv_, "wo": wo_,
            "wproj": wp_, "wup": wu_, "wdown": wd_,
            "consts": consts, "maskbig": maskbig.astype(bf),
        }
        if has_bias:
            m["ball"] = ball
        if nonzero_bv:
            m["bvmat"] = bvmat
        if nonzero_bv or nonzero_blog:
            m["e0_d"] = e0.astype(bf)
        if do_logits:
            m["wlog"] = wlog_bf
            if nonzero_blog:
                m["blogmat"] = blogmat
        in_maps.append(m)
    return in_maps, has_bias, nonzero_bv, nonzero_blog


_CACHE = {}


def _get_nc(key):
    if key not in _CACHE:
        _CACHE[key] = build(n_layers=4, do_logits=True, has_bias=key[0],
                            nonzero_bv=key[1], nonzero_blog=key[2])
    return _CACHE[key]


def kernel(**inputs):
    """Full-model entry: takes setup_inputs() arrays, returns [B, L, V] float32 logits."""
    from concourse.bass_utils import run_bass_kernel_spmd
    in_maps, hb, nzbv, nzbl = host_inputs(inputs, n_layers=4, do_logits=True)
    nc = _get_nc((hb, nzbv, nzbl))
    res = run_bass_kernel_spmd(nc, in_maps, core_ids=list(range(NCORE)))
    out = np.empty((B, L, V), np.float32)
    for c in range(NCORE):
        b, q = c // 4, c % 4
        out[b, q * T:(q + 1) * T, :] = res.results[c]["logits"]
    return np.ascontiguousarray(out, dtype=np.float32)


# revision 20
# speedup vs baseline: 1.0052x; 1.0052x over previous
# Bass/Tile kernel for nn_Decoder: 4-layer dense transformer, B=2 L=2048 D=1024 H=16 V=32000.
# Sharding: token-parallel over 8 cores (core c owns the 512-token quarter c%4 of batch c//4),
# per-layer K and V AllGathers (bf16) within batch groups [[0-3],[4-7]], token-sharded
# full-vocab logits (no final collective). Weights/KV/activations bf16, residual fp32.
# Causality via per-core 0/1 mask input data (program identical across cores - SPMD).
import contextlib
import numpy as np
import concourse.bass as bass
import concourse.mybir as mybir
import concourse.tile as tile
from concourse import bacc

P = 128
D = 1024
H = 16
DH = 64
FF = 2048
L = 2048
B = 2
V = 32000
NCORE = 8
T = 512            # own tokens per core
KO = D // P        # 8
FKO = FF // P      # 16
NKT = (4 * T) // P  # 16 key tiles (full batch)
VN = 500           # vocab N-tile width
NVT = V // VN      # 64
VROW = H * (DH + 1)  # 1040: v row with ones column per head
EPS = 1e-6
f32 = mybir.dt.float32
bf16 = mybir.dt.bfloat16
AF = mybir.ActivationFunctionType
ALU = mybir.AluOpType


def build(n_layers=4, do_logits=True, has_bias=False, nonzero_bv=False,
          nonzero_blog=False, want_xout=False):
    nc = bacc.Bacc(None, target_bir_lowering=False)

    x0T = nc.dram_tensor("x0T", [D, T], f32, kind="ExternalInput")
    wq = nc.dram_tensor("wq", [n_layers, D, D], bf16, kind="ExternalInput")
    wk = nc.dram_tensor("wk", [n_layers, D, D], bf16, kind="ExternalInput")
    wv = nc.dram_tensor("wv", [n_layers, D, D], bf16, kind="ExternalInput")
    wo = nc.dram_tensor("wo", [n_layers, D, D], bf16, kind="ExternalInput")
    wproj = nc.dram_tensor("wproj", [n_layers, D, FF], bf16, kind="ExternalInput")
    wup = nc.dram_tensor("wup", [n_layers, D, FF], bf16, kind="ExternalInput")
    wdown = nc.dram_tensor("wdown", [n_layers, FF, D], bf16, kind="ExternalInput")
    consts = nc.dram_tensor("consts", [P, 2 * P], bf16, kind="ExternalInput")
    # consts[:, 0:P] = ones; consts[:, P:P+2] used as bpat (col j<64 -> row0=1,
    # j>=64 -> row1=1) -- see host_inputs.
    maskbig = nc.dram_tensor("maskbig", [NKT, P, T], bf16, kind="ExternalInput")
    # bias columns: [bq 0:8 | bk 8:16 | bo 16:24 | bproj 24:40 | bup 40:56 | bdown 56:64]
    ball = (nc.dram_tensor("ball", [n_layers, P, 64], f32, kind="ExternalInput")
            if has_bias else None)
    bvmat = (nc.dram_tensor("bvmat", [n_layers, P, D], bf16, kind="ExternalInput")
             if nonzero_bv else None)
    e0_d = (nc.dram_tensor("e0_d", [P, P], bf16, kind="ExternalInput")
            if (nonzero_bv or nonzero_blog) else None)
    if do_logits:
        wlog = nc.dram_tensor("wlog", [D, V], bf16, kind="ExternalInput")
        blogmat = (nc.dram_tensor("blogmat", [P, V], bf16, kind="ExternalInput")
                   if nonzero_blog else None)
        logits = nc.dram_tensor("logits", [T, V], f32, kind="ExternalOutput")
    if want_xout:
        xout = nc.dram_tensor("xout", [P, KO, T], f32, kind="ExternalOutput")

    with tile.TileContext(nc) as tc, contextlib.ExitStack() as ctx:
        cn = ctx.enter_context(tc.tile_pool(name="cn", bufs=1))
        pb = ctx.enter_context(tc.tile_pool(name="pb", bufs=1))
        evn = ctx.enter_context(tc.tile_pool(name="evn", bufs=2))   # norm/small temps
        eva = ctx.enter_context(tc.tile_pool(name="eva", bufs=3))   # evict stream temps
        evf = ctx.enter_context(tc.tile_pool(name="evf", bufs=2))   # f32 evict temps
        exp_ = ctx.enter_context(tc.tile_pool(name="exp", bufs=6))  # exp'd score tiles
        aca = ctx.enter_context(tc.tile_pool(name="aca", bufs=2))   # attn acc evacs
        ps = ctx.enter_context(tc.tile_pool(name="ps", bufs=3, space="PSUM"))
        pa = ctx.enter_context(tc.tile_pool(name="pa", bufs=3, space="PSUM"))
        av = ctx.enter_context(tc.tile_pool(name="av", bufs=2, space="PSUM"))
        dr = ctx.enter_context(tc.tile_pool(name="dram", bufs=2, space="DRAM"))

        # ---- constants ----
        cst = cn.tile([P, 2 * P], bf16, tag="cst")
        nc.sync.dma_start(cst[:], consts[:])
        ones = cst[:, 0:P]           # [P, P] all-ones (bf16)
        bpat = cst[:, P:P + P]       # rows 0/1: head-pair broadcast pattern
        sc_rms = cn.tile([P, 1], f32, tag="sc_rms")
        nc.any.memset(sc_rms[:], 1.0 / D)
        eps_t = cn.tile([P, 1], f32, tag="eps")
        nc.any.memset(eps_t[:], EPS)
        sc_att = cn.tile([P, 1], f32, tag="sc_att")
        nc.any.memset(sc_att[:], 0.125)
        e0 = None
        if e0_d is not None:
            e0 = cn.tile([P, P], bf16, tag="e0")
            nc.sync.dma_start(e0[:], e0_d[:])
        maskS = []
        for kt in range(NKT):
            m = cn.tile([P, T], bf16, tag=f"mask{kt}")
            nc.sync.dma_start(m[:], maskbig[kt])
            maskS.append(m)

        # ---- persistent activations ----
        xT = pb.tile([P, KO, T], f32, tag="xT")
        QT = pb.tile([P, KO, T], bf16, tag="QT")

        # ---- Phase 0: load xT (host already did embed+pe+transpose) ----
        nc.sync.dma_start(xT[:], x0T.rearrange("(ko p) t -> p ko t", p=P))

        def rmsnorm(nm):
            normT = pb.tile([P, KO, T], bf16, tag="nta", name=nm)
            ssq = ps.tile([P, T], f32, tag="mm512")
            for ko in range(KO):
                x2 = evn.tile([P, T], bf16, tag="x2")
                nc.vector.tensor_tensor(x2[:], xT[:, ko], xT[:, ko], ALU.mult)
                nc.tensor.matmul(ssq[:], ones[:], x2[:], start=(ko == 0),
                                 stop=(ko == KO - 1))
            srt = evn.tile([P, T], f32, tag="srt")
            nc.scalar.activation(srt[:], ssq[:], AF.Sqrt, bias=eps_t[:], scale=sc_rms[:])
            inv = evn.tile([P, T], f32, tag="inv")
            nc.vector.reciprocal(inv[:], srt[:])
            nc.vector.tensor_tensor(
                normT[:], xT[:], inv[:, None, :].to_broadcast([P, KO, T]), ALU.mult)
            return normT

        with tc.tile_pool(name="wp", bufs=3) as wp, \
             tc.tile_pool(name="wdp", bufs=2) as wdp, \
             tc.tile_pool(name="kvp", bufs=1) as kvp, \
             tc.tile_pool(name="acp", bufs=1) as acp:

            def proj_T(w_, i, src, bias_col, consume, n_mt=KO, n_ko=KO):
                w3 = w_[i].rearrange("(ko p) m -> p ko m", p=P)
                for m in range(n_mt):
                    wt = wp.tile([P, n_ko, P], bf16, tag="wlhs")
                    nc.sync.dma_start(wt[:], w3[:, :, m * P:(m + 1) * P])
                    pt = ps.tile([P, T], f32, tag="mm512")
                    for ko in range(n_ko):
                        nc.tensor.matmul(pt[:], wt[:, ko], src[:, ko], start=(ko == 0),
                                         stop=(ko == n_ko - 1))
                    consume(m, pt, bias_col[:, m:m + 1] if bias_col is not None else None)

            for li in range(n_layers):
                bias_t = None
                if has_bias:
                    bias_t = evn.tile([P, 64], f32, tag="bias")
                    nc.sync.dma_start(bias_t[:], ball[li])

                normT = rmsnorm(f"norm1_{li}")

                # AG1 carries K + V-heads-0-7; AG2 carries V-heads-8-15, so the
                # first half of attention never waits on the second collective.
                HVROW = (H // 2) * (DH + 1)  # 520
                ag1_in = dr.tile([D * T + T * HVROW], bf16, tag="ag1_in")
                ag1_out = dr.tile([4, D * T + T * HVROW], bf16, tag="ag1_out")
                ag2_in = dr.tile([T * HVROW], bf16, tag="ag2_in")
                ag2_out = dr.tile([4, T * HVROW], bf16, tag="ag2_out")
                kT_view = ag1_in[0:D * T].rearrange("(d t) -> d t", t=T)
                vv1 = ag1_in[D * T:].rearrange("(t h d) -> t h d", h=H // 2, d=DH + 1)
                vv2 = ag2_in.rearrange("(t h d) -> t h d", h=H // 2, d=DH + 1)

                def k_consume(m, pt, bcol, kT_view=kT_view):
                    kt_sb = eva.tile([P, T], bf16, tag="eva")
                    if bcol is not None:
                        nc.scalar.activation(kt_sb[:], pt[:], AF.Identity, bias=bcol[:])
                    else:
                        nc.vector.tensor_copy(kt_sb[:], pt[:])
                    nc.sync.dma_start(kT_view[m * P:(m + 1) * P, :], kt_sb[:])
                proj_T(wk, li, normT, bias_t[:, 8:16] if has_bias else None, k_consume)

                # ---- V projection (natural [T, D] layout, weight as moving op) ----
                wv3 = wv[li].rearrange("(ko p) m -> ko p m", p=P)
                for nh in range(D // T):
                    vv = vv1 if nh == 0 else vv2
                    for mp in range(2):  # token-tile pairs share the weight stream
                        pts = [ps.tile([P, T], f32, tag="mm512", name=f"vpt{i}") for i in range(2)]
                        for ko in range(KO):
                            wt = wp.tile([P, T], bf16, tag="wrhs")
                            nc.sync.dma_start(wt[:], wv3[ko, :, nh * T:(nh + 1) * T])
                            for mi in range(2):
                                mt = 2 * mp + mi
                                last = (ko == KO - 1) and bvmat is None
                                nc.tensor.matmul(
                                    pts[mi][:], normT[:, ko, mt * P:(mt + 1) * P],
                                    wt[:], start=(ko == 0), stop=last)
                        for mi in range(2):
                            mt = 2 * mp + mi
                            if bvmat is not None:
                                bvt = wp.tile([P, T], bf16, tag="wrhs")
                                nc.sync.dma_start(bvt[:],
                                                  bvmat[li][:, nh * T:(nh + 1) * T])
                                nc.tensor.matmul(pts[mi][:], e0[:, :P], bvt[:],
                                                 start=False, stop=True)
                            v_sb = eva.tile([P, T], bf16, tag="eva")
                            nc.vector.tensor_copy(v_sb[:], pts[mi][:])
                            nc.sync.dma_start(
                                vv[mt * P:(mt + 1) * P, :, 0:DH],
                                v_sb.rearrange("p (h d) -> p h d", d=DH))
                    # ones column for the denominator rows of this nh's heads
                    on = eva.tile([P, KO], bf16, tag="evon")
                    nc.vector.tensor_copy(on[:], ones[:, 0:KO])
                    for tt in range(T // P):
                        nc.sync.dma_start(
                            vv[tt * P:(tt + 1) * P, :, DH:DH + 1]
                            .rearrange("p h d -> p (h d)"),
                            on[:])
                    if nh == 0:
                        nc.gpsimd.collective_compute(
                            "AllGather", ALU.bypass, ins=[ag1_in[:]],
                            outs=[ag1_out[:]],
                            replica_groups=[[0, 1, 2, 3], [4, 5, 6, 7]])
                    else:
                        nc.gpsimd.collective_compute(
                            "AllGather", ALU.bypass, ins=[ag2_in[:]],
                            outs=[ag2_out[:]],
                            replica_groups=[[0, 1, 2, 3], [4, 5, 6, 7]])

                # ---- attention ----
                # AG1-sourced tiles first on the DMA queue (K + V heads 0-7),
                # AG2-sourced (V heads 8-15) after - no head-of-line blocking.
                # Issued before the Q projection so the cache DMAs aren't
                # queued behind Q's weight loads.
                ktt, vt1, vt2 = [], [], []
                for kt in range(NKT):
                    g, off = kt // 4, (kt % 4) * P
                    kth = kvp.tile([P, KO, P], bf16, tag=f"kth{kt}", name=f"kth{kt}")
                    nc.sync.dma_start(
                        kth[:],
                        ag1_out[g, 0:D * T].rearrange("(ko p t) -> p ko t", p=P, t=T)
                        [:, :, off:off + P])
                    ktt.append(kth)
                for kt in range(NKT):
                    g, off = kt // 4, (kt % 4) * P
                    vth = kvp.tile([P, H // 2, DH + 1], bf16, tag=f"vth{kt}",
                                   name=f"vth{kt}")
                    nc.sync.dma_start(
                        vth[:],
                        ag1_out[g, D * T:].rearrange("(t h d) -> t h d", h=H // 2,
                                                     d=DH + 1)[off:off + P])
                    vt1.append(vth)
                for kt in range(NKT):
                    g, off = kt // 4, (kt % 4) * P
                    vth = kvp.tile([P, H // 2, DH + 1], bf16, tag=f"vth2_{kt}",
                                   name=f"vth2_{kt}")
                    nc.sync.dma_start(
                        vth[:],
                        ag2_out[g].rearrange("(t h d) -> t h d", h=H // 2,
                                             d=DH + 1)[off:off + P])
                    vt2.append(vth)

                def q_consume(m, pt, bcol):
                    if bcol is not None:
                        nc.scalar.activation(QT[:, m, :], pt[:], AF.Identity, bias=bcol[:])
                    else:
                        nc.vector.tensor_copy(QT[:, m, :], pt[:])
                proj_T(wq, li, normT, bias_t[:, 0:8] if has_bias else None, q_consume)

                aoT = pb.tile([P, KO, T], bf16, tag="nta", name=f"aoT_{li}")
                DEPTH = 2  # sw-pipeline: AV for kt lags its score by DEPTH tiles
                for hko in range(KO):
                    avp = [av.tile([DH + 1, T], f32, tag="avacc", name=f"avp{i}") for i in range(2)]
                    exts = {}
                    for kt in range(NKT + DEPTH):
                        if kt < NKT:
                            for hi in range(2):
                                pbase = DH * hi
                                sp = pa.tile([P, T], f32, tag="att")
                                nc.tensor.matmul(sp[:],
                                                 ktt[kt][pbase:pbase + DH, hko, :],
                                                 QT[pbase:pbase + DH, hko, :],
                                                 start=True, stop=True,
                                                 tile_position=(pbase, 0))
                                ext = exp_.tile([P, T], bf16, tag="exs")
                                nc.scalar.activation(ext[:], sp[:], AF.Exp,
                                                     scale=sc_att[:])
                                nc.vector.tensor_tensor(ext[:], ext[:], maskS[kt][:],
                                                        ALU.mult)
                                exts[(kt, hi)] = ext
                        if kt >= DEPTH:
                            k2 = kt - DEPTH
                            for hi in range(2):
                                h = 2 * hko + hi
                                vsl = (vt1[k2][:, h, :] if h < H // 2
                                       else vt2[k2][:, h - H // 2, :])
                                nc.tensor.matmul(avp[hi][:], vsl, exts.pop((k2, hi)),
                                                 start=(k2 == 0),
                                                 stop=(k2 == NKT - 1))
                    # softmax denominators + normalization for this head pair
                    for hi in range(2):
                        ac = aca.tile([DH + 1, T], bf16, tag=f"ac{hi}", name=f"ac{hi}")
                        nc.vector.tensor_copy(ac[:], avp[hi][:])
                        invd = evn.tile([DH + 1, T], f32, tag="invd")
                        nc.vector.reciprocal(invd[DH:DH + 1, :], avp[hi][DH:DH + 1, :])
                        invdr = evn.tile([DH + 1, T], bf16, tag="invdr")
                        nc.vector.tensor_copy(invdr[DH:DH + 1, :], invd[DH:DH + 1, :])
                        bcp = pa.tile([DH, T], f32, tag="att", name="bcp")
                        nc.tensor.matmul(bcp[:], ones[DH:DH + 1, 0:DH],
                                         invdr[DH:DH + 1, :], start=True, stop=True)
                        invb = eva.tile([DH, T], bf16, tag="invb")
                        nc.vector.tensor_copy(invb[:], bcp[:])
                        pbase = DH * hi
                        nc.vector.tensor_tensor(aoT[pbase:pbase + DH, hko, :],
                                                ac[0:DH, :], invb[:], ALU.mult)

                # ---- O projection + residual ----
                def o_consume(m, pt, bcol):
                    ot = evf.tile([P, T], f32, tag="evf")
                    if bcol is not None:
                        nc.scalar.activation(ot[:], pt[:], AF.Identity, bias=bcol[:])
                    else:
                        nc.vector.tensor_copy(ot[:], pt[:])
                    nc.vector.tensor_tensor(xT[:, m, :], xT[:, m, :], ot[:], ALU.add)
                proj_T(wo, li, aoT, bias_t[:, 16:24] if has_bias else None, o_consume)

                # ---- FFN ----
                normT = rmsnorm(f"norm2_{li}")
                hts = []
                wp3 = wproj[li].rearrange("(ko p) m -> p ko m", p=P)
                wu3 = wup[li].rearrange("(ko p) m -> p ko m", p=P)
                for m in range(FKO):
                    wtp = wp.tile([P, KO, P], bf16, tag="wlhs")
                    nc.sync.dma_start(wtp[:], wp3[:, :, m * P:(m + 1) * P])
                    ptp = ps.tile([P, T], f32, tag="mm512")
                    for ko in range(KO):
                        nc.tensor.matmul(ptp[:], wtp[:, ko], normT[:, ko], start=(ko == 0),
                                         stop=(ko == KO - 1))
                    wtu = wp.tile([P, KO, P], bf16, tag="wlhs")
                    nc.sync.dma_start(wtu[:], wu3[:, :, m * P:(m + 1) * P])
                    ptu = ps.tile([P, T], f32, tag="mm512")
                    for ko in range(KO):
                        nc.tensor.matmul(ptu[:], wtu[:, ko], normT[:, ko], start=(ko == 0),
                                         stop=(ko == KO - 1))
                    usb = evn.tile([P, T], f32, tag="uev")
                    if has_bias:
                        nc.scalar.activation(usb[:], ptu[:], AF.Identity,
                                             bias=bias_t[:, 40 + m:41 + m])
                        psb = evn.tile([P, T], f32, tag="pev")
                        nc.scalar.activation(psb[:], ptp[:], AF.Identity,
                                             bias=bias_t[:, 24 + m:25 + m])
                        gsb = evn.tile([P, T], bf16, tag="gev")
                        nc.vector.tensor_tensor(gsb[:], psb[:], usb[:], ALU.mult)
                    else:
                        nc.scalar.activation(usb[:], ptu[:], AF.Copy)
                        gsb = evn.tile([P, T], bf16, tag="gev")
                        nc.vector.tensor_tensor(gsb[:], ptp[:], usb[:], ALU.mult)
                    sgt = evn.tile([P, T], bf16, tag="sgt")
                    nc.scalar.activation(sgt[:], gsb[:], AF.Sigmoid)
                    ht = acp.tile([P, T], bf16, tag=f"acc{m}", name=f"ht{m}")
                    nc.vector.tensor_tensor(ht[:], gsb[:], sgt[:], ALU.mult)
                    hts.append(ht)
                wd3 = wdown[li].rearrange("(ko p) m -> p ko m", p=P)
                for m in range(KO):
                    wt = wdp.tile([P, FKO, P], bf16, tag="wdn")
                    nc.sync.dma_start(wt[:], wd3[:, :, m * P:(m + 1) * P])
                    pt = ps.tile([P, T], f32, tag="mm512")
                    for ko in range(FKO):
                        nc.tensor.matmul(pt[:], wt[:, ko], hts[ko][:], start=(ko == 0),
                                         stop=(ko == FKO - 1))
                    dt_ = evf.tile([P, T], f32, tag="evf")
                    if has_bias:
                        nc.scalar.activation(dt_[:], pt[:], AF.Identity,
                                             bias=bias_t[:, 56 + m:57 + m])
                    else:
                        nc.vector.tensor_copy(dt_[:], pt[:])
                    nc.vector.tensor_tensor(xT[:, m, :], xT[:, m, :], dt_[:], ALU.add)

        if want_xout:
            nc.sync.dma_start(xout[:], xT[:])

        if do_logits:
            xb = pb.tile([P, KO, T], bf16, tag="QT")
            for ko in range(KO):  # per-ko: each unblocks as layer-3 FFN finishes
                nc.vector.tensor_copy(xb[:, ko], xT[:, ko])
            wl3 = wlog.rearrange("(ko p) m -> p ko m", p=P)
            with tc.tile_pool(name="wlp", bufs=3) as wlp, \
                 tc.tile_pool(name="lev", bufs=3) as lev:
                for nt in range(NVT):
                    wt = wlp.tile([P, KO, VN], bf16, tag="wlog")
                    nc.sync.dma_start(wt[:], wl3[:, :, nt * VN:(nt + 1) * VN])
                    blt = None
                    if blogmat is not None:
                        blt = wlp.tile([P, VN], bf16, tag="wlogb")
                        nc.sync.dma_start(blt[:], blogmat[:, nt * VN:(nt + 1) * VN])
                    for mt in range(T // P):
                        pt = ps.tile([P, VN], f32, tag="mm512")
                        for ko in range(KO):
                            last = (ko == KO - 1) and blogmat is None
                            nc.tensor.matmul(pt[:], xb[:, ko, mt * P:(mt + 1) * P],
                                             wt[:, ko], start=(ko == 0), stop=last)
                        if blt is not None:
                            nc.tensor.matmul(pt[:], e0[:, :P], blt[:],
                                             start=False, stop=True)
                        lo = lev.tile([P, VN], f32, tag="evl")
                        nc.vector.tensor_copy(lo[:], pt[:])
                        nc.sync.dma_start(
                            logits[mt * P:(mt + 1) * P, nt * VN:(nt + 1) * VN], lo[:])

    nc.compile()
    return nc


def host_inputs(inp, n_layers=4, do_logits=True):
    """Build per-core in_maps from the full model inputs dict (numpy)."""
    import math
    import ml_dtypes
    bf = ml_dtypes.bfloat16
    g = {k: np.asarray(v) for k, v in inp.items()}
    ids = g["input_ids"].astype(np.int64)
    embed = g["embed"].astype(np.float32)
    pos = np.arange(L, dtype=np.float32)[:, None]
    div = np.exp(np.arange(0, D, 2, dtype=np.float32) * (-math.log(10000.0) / D))
    ang = pos * div
    pe = np.zeros((L, D), dtype=np.float32)
    pe[:, 0::2] = np.sin(ang)
    pe[:, 1::2] = np.cos(ang)

    gam = g["gammas"].astype(np.float32)
    wq_ = np.ascontiguousarray(gam[0:2 * n_layers:2, :, None] * g["Wq"][:n_layers]).astype(bf)
    wk_ = np.ascontiguousarray(gam[0:2 * n_layers:2, :, None] * g["Wk"][:n_layers]).astype(bf)
    wv_ = np.ascontiguousarray(gam[0:2 * n_layers:2, :, None] * g["Wv"][:n_layers]).astype(bf)
    wo_ = np.ascontiguousarray(g["Wo"][:n_layers]).astype(bf)
    wp_ = np.ascontiguousarray(gam[1:2 * n_layers:2, :, None] * g["Wproj"][:n_layers]).astype(bf)
    wu_ = np.ascontiguousarray(gam[1:2 * n_layers:2, :, None] * g["Wup"][:n_layers]).astype(bf)
    wd_ = np.ascontiguousarray(g["Wdown"][:n_layers]).astype(bf)

    has_bias = bool(
        np.any(g["bq"][:n_layers]) or np.any(g["bk"][:n_layers])
        or np.any(g["bo"][:n_layers]) or np.any(g["bproj"][:n_layers])
        or np.any(g["bup"][:n_layers]) or np.any(g["bdown"][:n_layers]))
    nonzero_bv = bool(np.any(g["bv"][:n_layers] != 0))
    nonzero_blog = do_logits and bool(np.any(g["blogits"] != 0))

    ball = None
    if has_bias:
        ball = np.zeros((n_layers, P, 64), np.float32)
        for i in range(n_layers):
            ball[i, :, 0:8] = g["bq"][i].reshape(8, P).T
            ball[i, :, 8:16] = g["bk"][i].reshape(8, P).T
            ball[i, :, 16:24] = g["bo"][i].reshape(8, P).T
            ball[i, :, 24:40] = g["bproj"][i].reshape(16, P).T
            ball[i, :, 40:56] = g["bup"][i].reshape(16, P).T
            ball[i, :, 56:64] = g["bdown"][i].reshape(8, P).T
    bvmat = None
    if nonzero_bv:
        bvmat = np.zeros((n_layers, P, D), np.float32)
        bvmat[:, 0, :] = g["bv"][:n_layers]
        bvmat = bvmat.astype(bf)
    e0 = np.zeros((P, P), np.float32)
    e0[0, :] = 1.0

    consts = np.zeros((P, 2 * P), np.float32)
    consts[:, 0:P] = 1.0
    consts[0, P:P + DH] = 1.0        # bpat row 0 -> out partitions 0:64
    consts[1, P + DH:P + P] = 1.0    # bpat row 1 -> out partitions 64:128
    consts = consts.astype(bf)

    wlog_bf = None
    if do_logits:
        wlog_bf = np.ascontiguousarray(g["Wlogits"]).astype(bf)
        blogmat = None
        if nonzero_blog:
            blogmat = np.zeros((P, V), np.float32)
            blogmat[0, :] = g["blogits"]
            blogmat = blogmat.astype(bf)

    in_maps = []
    for c in range(NCORE):
        b, q = c // 4, c % 4
        tok = ids[b, q * T:(q + 1) * T]
        x0 = embed[tok] + pe[q * T:(q + 1) * T, :]
        i_idx = np.arange(P)[:, None]
        j_idx = np.arange(T)[None, :]
        maskbig = np.zeros((NKT, P, T), np.float32)
        for kt in range(NKT):
            maskbig[kt] = ((kt * P + i_idx) <= (q * T + j_idx)).astype(np.float32)
        m = {
            "x0T": np.ascontiguousarray(x0.T, np.float32),
            "wq": wq_, "wk": wk_, "wv": wv_, "wo": wo_,
            "wproj": wp_, "wup": wu_, "wdown": wd_,
            "consts": consts, "maskbig": maskbig.astype(bf),
        }
        if has_bias:
            m["ball"] = ball
        if nonzero_bv:
            m["bvmat"] = bvmat
        if nonzero_bv or nonzero_blog:
            m["e0_d"] = e0.astype(bf)
        if do_logits:
            m["wlog"] = wlog_bf
            if nonzero_blog:
                m["blogmat"] = blogmat
        in_maps.append(m)
    return in_maps, has_bias, nonzero_bv, nonzero_blog


_CACHE = {}


def _get_nc(key):
    if key not in _CACHE:
        _CACHE[key] = build(n_layers=4, do_logits=True, has_bias=key[0],
                            nonzero_bv=key[1], nonzero_blog=key[2])
    return _CACHE[key]


def kernel(**inputs):
    """Full-model entry: takes setup_inputs() arrays, returns [B, L, V] float32 logits."""
    from concourse.bass_utils import run_bass_kernel_spmd
    in_maps, hb, nzbv, nzbl = host_inputs(inputs, n_layers=4, do_logits=True)
    nc = _get_nc((hb, nzbv, nzbl))
    res = run_bass_kernel_spmd(nc, in_maps, core_ids=list(range(NCORE)))
    out = np.empty((B, L, V), np.float32)
    for c in range(NCORE):
        b, q = c // 4, c % 4
        out[b, q * T:(q + 1) * T, :] = res.results[c]["logits"]
    return np.ascontiguousarray(out, dtype=np.float32)


# revision 22
# speedup vs baseline: 1.0693x; 1.0637x over previous
# Bass/Tile kernel for nn_Decoder: 4-layer dense transformer, B=2 L=2048 D=1024 H=16 V=32000.
# Sharding: token-parallel over 8 cores (core c owns the 512-token quarter c%4 of batch c//4),
# per-layer K and V AllGathers (bf16) within batch groups [[0-3],[4-7]], token-sharded
# full-vocab logits (no final collective). Weights/KV/activations bf16, residual fp32.
# Causality via per-core 0/1 mask input data (program identical across cores - SPMD).
import contextlib
import numpy as np
import concourse.bass as bass
import concourse.mybir as mybir
import concourse.tile as tile
from concourse import bacc

P = 128
D = 1024
H = 16
DH = 64
FF = 2048
L = 2048
B = 2
V = 32000
NCORE = 8
T = 512            # own tokens per core
KO = D // P        # 8
FKO = FF // P      # 16
NKT = (4 * T) // P  # 16 key tiles (full batch)
VN = 500           # vocab N-tile width
NVT = V // VN      # 64
VROW = H * (DH + 1)  # 1040: v row with ones column per head
EPS = 1e-6
f32 = mybir.dt.float32
bf16 = mybir.dt.bfloat16
AF = mybir.ActivationFunctionType
ALU = mybir.AluOpType


def build(n_layers=4, do_logits=True, has_bias=False, nonzero_bv=False,
          nonzero_blog=False, want_xout=False):
    nc = bacc.Bacc(None, target_bir_lowering=False)

    x0T = nc.dram_tensor("x0T", [D, T], f32, kind="ExternalInput")
    wq = nc.dram_tensor("wq", [n_layers, D, D], bf16, kind="ExternalInput")
    wk = nc.dram_tensor("wk", [n_layers, D, D], bf16, kind="ExternalInput")
    wv = nc.dram_tensor("wv", [n_layers, D, D], bf16, kind="ExternalInput")
    wo = nc.dram_tensor("wo", [n_layers, D, D], bf16, kind="ExternalInput")
    wproj = nc.dram_tensor("wproj", [n_layers, D, FF], bf16, kind="ExternalInput")
    wup = nc.dram_tensor("wup", [n_layers, D, FF], bf16, kind="ExternalInput")
    wdown = nc.dram_tensor("wdown", [n_layers, FF, D], bf16, kind="ExternalInput")
    consts = nc.dram_tensor("consts", [P, 2 * P], bf16, kind="ExternalInput")
    # consts[:, 0:P] = ones; consts[:, P:P+2] used as bpat (col j<64 -> row0=1,
    # j>=64 -> row1=1) -- see host_inputs.
    maskbig = nc.dram_tensor("maskbig", [NKT, P, T], bf16, kind="ExternalInput")
    # bias columns: [bq 0:8 | bk 8:16 | bo 16:24 | bproj 24:40 | bup 40:56 | bdown 56:64]
    ball = (nc.dram_tensor("ball", [n_layers, P, 64], f32, kind="ExternalInput")
            if has_bias else None)
    bvmat = (nc.dram_tensor("bvmat", [n_layers, P, D], bf16, kind="ExternalInput")
             if nonzero_bv else None)
    e0_d = (nc.dram_tensor("e0_d", [P, P], bf16, kind="ExternalInput")
            if (nonzero_bv or nonzero_blog) else None)
    if do_logits:
        wlog = nc.dram_tensor("wlog", [D, V], bf16, kind="ExternalInput")
        blogmat = (nc.dram_tensor("blogmat", [P, V], bf16, kind="ExternalInput")
                   if nonzero_blog else None)
        logits = nc.dram_tensor("logits", [T, V], f32, kind="ExternalOutput")
    if want_xout:
        xout = nc.dram_tensor("xout", [P, KO, T], f32, kind="ExternalOutput")

    with tile.TileContext(nc) as tc, contextlib.ExitStack() as ctx:
        cn = ctx.enter_context(tc.tile_pool(name="cn", bufs=1))
        pb = ctx.enter_context(tc.tile_pool(name="pb", bufs=1))
        evn = ctx.enter_context(tc.tile_pool(name="evn", bufs=2))   # norm/small temps
        eva = ctx.enter_context(tc.tile_pool(name="eva", bufs=3))   # evict stream temps
        evf = ctx.enter_context(tc.tile_pool(name="evf", bufs=2))   # f32 evict temps
        exp_ = ctx.enter_context(tc.tile_pool(name="exp", bufs=3))  # exp'd score tiles
        aca = ctx.enter_context(tc.tile_pool(name="aca", bufs=2))   # attn acc evacs
        ps = ctx.enter_context(tc.tile_pool(name="ps", bufs=3, space="PSUM"))
        pa = ctx.enter_context(tc.tile_pool(name="pa", bufs=3, space="PSUM"))
        av = ctx.enter_context(tc.tile_pool(name="av", bufs=2, space="PSUM"))
        dr = ctx.enter_context(tc.tile_pool(name="dram", bufs=2, space="DRAM"))

        # ---- constants ----
        cst = cn.tile([P, 2 * P], bf16, tag="cst")
        nc.sync.dma_start(cst[:], consts[:])
        ones = cst[:, 0:P]           # [P, P] all-ones (bf16)
        bpat = cst[:, P:P + P]       # rows 0/1: head-pair broadcast pattern
        sc_rms = cn.tile([P, 1], f32, tag="sc_rms")
        nc.any.memset(sc_rms[:], 1.0 / D)
        eps_t = cn.tile([P, 1], f32, tag="eps")
        nc.any.memset(eps_t[:], EPS)
        sc_att = cn.tile([P, 1], f32, tag="sc_att")
        nc.any.memset(sc_att[:], 0.125)
        e0 = None
        if e0_d is not None:
            e0 = cn.tile([P, P], bf16, tag="e0")
            nc.sync.dma_start(e0[:], e0_d[:])
        maskS = []
        for kt in range(NKT):
            m = cn.tile([P, T], bf16, tag=f"mask{kt}")
            nc.sync.dma_start(m[:], maskbig[kt])
            maskS.append(m)

        # ---- persistent activations ----
        xT = pb.tile([P, KO, T], f32, tag="xT")
        QT = pb.tile([P, KO, T], bf16, tag="QT")

        # ---- Phase 0: load xT (host already did embed+pe+transpose) ----
        nc.sync.dma_start(xT[:], x0T.rearrange("(ko p) t -> p ko t", p=P))

        def rmsnorm(nm):
            normT = pb.tile([P, KO, T], bf16, tag="nta", name=nm)
            ssq = ps.tile([P, T], f32, tag="mm512")
            for ko in range(KO):
                x2 = evn.tile([P, T], bf16, tag="x2")
                nc.vector.tensor_tensor(x2[:], xT[:, ko], xT[:, ko], ALU.mult)
                nc.tensor.matmul(ssq[:], ones[:], x2[:], start=(ko == 0),
                                 stop=(ko == KO - 1))
            srt = evn.tile([P, T], f32, tag="srt")
            nc.scalar.activation(srt[:], ssq[:], AF.Sqrt, bias=eps_t[:], scale=sc_rms[:])
            inv = evn.tile([P, T], f32, tag="inv")
            nc.vector.reciprocal(inv[:], srt[:])
            nc.vector.tensor_tensor(
                normT[:], xT[:], inv[:, None, :].to_broadcast([P, KO, T]), ALU.mult)
            return normT

        with tc.tile_pool(name="wp", bufs=3) as wp, \
             tc.tile_pool(name="wdp", bufs=2) as wdp, \
             tc.tile_pool(name="kvp", bufs=1) as kvp, \
             tc.tile_pool(name="acp", bufs=1) as acp:

            def proj_T(w_, i, src, bias_col, consume, n_mt=KO, n_ko=KO):
                w3 = w_[i].rearrange("(ko p) m -> p ko m", p=P)
                for m in range(n_mt):
                    wt = wp.tile([P, n_ko, P], bf16, tag="wlhs")
                    nc.sync.dma_start(wt[:], w3[:, :, m * P:(m + 1) * P])
                    pt = ps.tile([P, T], f32, tag="mm512")
                    for ko in range(n_ko):
                        nc.tensor.matmul(pt[:], wt[:, ko], src[:, ko], start=(ko == 0),
                                         stop=(ko == n_ko - 1))
                    consume(m, pt, bias_col[:, m:m + 1] if bias_col is not None else None)

            for li in range(n_layers):
                bias_t = None
                if has_bias:
                    bias_t = evn.tile([P, 64], f32, tag="bias")
                    nc.sync.dma_start(bias_t[:], ball[li])

                normT = rmsnorm(f"norm1_{li}")

                # AG1 carries K + V-heads-0-7; AG2 carries V-heads-8-15, so the
                # first half of attention never waits on the second collective.
                HVROW = (H // 2) * (DH + 1)  # 520
                ag1_in = dr.tile([D * T + T * HVROW], bf16, tag="ag1_in")
                ag1_out = dr.tile([4, D * T + T * HVROW], bf16, tag="ag1_out")
                ag2_in = dr.tile([T * HVROW], bf16, tag="ag2_in")
                ag2_out = dr.tile([4, T * HVROW], bf16, tag="ag2_out")
                kT_view = ag1_in[0:D * T].rearrange("(d t) -> d t", t=T)
                vv1 = ag1_in[D * T:].rearrange("(t h d) -> t h d", h=H // 2, d=DH + 1)
                vv2 = ag2_in.rearrange("(t h d) -> t h d", h=H // 2, d=DH + 1)

                def k_consume(m, pt, bcol, kT_view=kT_view):
                    kt_sb = eva.tile([P, T], bf16, tag="eva")
                    if bcol is not None:
                        nc.scalar.activation(kt_sb[:], pt[:], AF.Identity, bias=bcol[:])
                    else:
                        nc.vector.tensor_copy(kt_sb[:], pt[:])
                    nc.sync.dma_start(kT_view[m * P:(m + 1) * P, :], kt_sb[:])
                proj_T(wk, li, normT, bias_t[:, 8:16] if has_bias else None, k_consume)

                # ---- V projection (natural [T, D] layout, weight as moving op) ----
                wv3 = wv[li].rearrange("(ko p) m -> ko p m", p=P)
                for nh in range(D // T):
                    vv = vv1 if nh == 0 else vv2
                    for mp in range(2):  # token-tile pairs share the weight stream
                        pts = [ps.tile([P, T], f32, tag="mm512", name=f"vpt{i}") for i in range(2)]
                        for ko in range(KO):
                            wt = wp.tile([P, T], bf16, tag="wrhs")
                            nc.sync.dma_start(wt[:], wv3[ko, :, nh * T:(nh + 1) * T])
                            for mi in range(2):
                                mt = 2 * mp + mi
                                last = (ko == KO - 1) and bvmat is None
                                nc.tensor.matmul(
                                    pts[mi][:], normT[:, ko, mt * P:(mt + 1) * P],
                                    wt[:], start=(ko == 0), stop=last)
                        for mi in range(2):
                            mt = 2 * mp + mi
                            if bvmat is not None:
                                bvt = wp.tile([P, T], bf16, tag="wrhs")
                                nc.sync.dma_start(bvt[:],
                                                  bvmat[li][:, nh * T:(nh + 1) * T])
                                nc.tensor.matmul(pts[mi][:], e0[:, :P], bvt[:],
                                                 start=False, stop=True)
                            v_sb = eva.tile([P, T], bf16, tag="eva")
                            nc.vector.tensor_copy(v_sb[:], pts[mi][:])
                            nc.sync.dma_start(
                                vv[mt * P:(mt + 1) * P, :, 0:DH],
                                v_sb.rearrange("p (h d) -> p h d", d=DH))
                    # ones column for the denominator rows of this nh's heads
                    on = eva.tile([P, KO], bf16, tag="evon")
                    nc.vector.tensor_copy(on[:], ones[:, 0:KO])
                    for tt in range(T // P):
                        nc.sync.dma_start(
                            vv[tt * P:(tt + 1) * P, :, DH:DH + 1]
                            .rearrange("p h d -> p (h d)"),
                            on[:])
                    if nh == 0:
                        nc.gpsimd.collective_compute(
                            "AllGather", ALU.bypass, ins=[ag1_in[:]],
                            outs=[ag1_out[:]],
                            replica_groups=[[0, 1, 2, 3], [4, 5, 6, 7]])
                    else:
                        nc.gpsimd.collective_compute(
                            "AllGather", ALU.bypass, ins=[ag2_in[:]],
                            outs=[ag2_out[:]],
                            replica_groups=[[0, 1, 2, 3], [4, 5, 6, 7]])

                def q_consume(m, pt, bcol):
                    if bcol is not None:
                        nc.scalar.activation(QT[:, m, :], pt[:], AF.Identity, bias=bcol[:])
                    else:
                        nc.vector.tensor_copy(QT[:, m, :], pt[:])
                proj_T(wq, li, normT, bias_t[:, 0:8] if has_bias else None, q_consume)

                # ---- attention ----
                # AG1-sourced tiles first on the DMA queue (K + V heads 0-7),
                # AG2-sourced (V heads 8-15) after - no head-of-line blocking.
                # Issued before the Q projection so the cache DMAs aren't
                # queued behind Q's weight loads.
                ktt, vt1, vt2 = [], [], []
                for kt in range(NKT):
                    g, off = kt // 4, (kt % 4) * P
                    kth = kvp.tile([P, KO, P], bf16, tag=f"kth{kt}", name=f"kth{kt}")
                    nc.sync.dma_start(
                        kth[:],
                        ag1_out[g, 0:D * T].rearrange("(ko p t) -> p ko t", p=P, t=T)
                        [:, :, off:off + P])
                    ktt.append(kth)
                for kt in range(NKT):
                    g, off = kt // 4, (kt % 4) * P
                    vth = kvp.tile([P, H // 2, DH + 1], bf16, tag=f"vth{kt}",
                                   name=f"vth{kt}")
                    nc.sync.dma_start(
                        vth[:],
                        ag1_out[g, D * T:].rearrange("(t h d) -> t h d", h=H // 2,
                                                     d=DH + 1)[off:off + P])
                    vt1.append(vth)
                for kt in range(NKT):
                    g, off = kt // 4, (kt % 4) * P
                    vth = kvp.tile([P, H // 2, DH + 1], bf16, tag=f"vth2_{kt}",
                                   name=f"vth2_{kt}")
                    nc.sync.dma_start(
                        vth[:],
                        ag2_out[g].rearrange("(t h d) -> t h d", h=H // 2,
                                             d=DH + 1)[off:off + P])
                    vt2.append(vth)

                aoT = pb.tile([P, KO, T], bf16, tag="nta", name=f"aoT_{li}")
                for hko in range(KO):
                    avp = [av.tile([DH + 1, T], f32, tag="avacc", name=f"avp{i}") for i in range(2)]
                    for kt in range(NKT):
                        for hi in range(2):
                            h = 2 * hko + hi
                            pbase = DH * hi
                            sp = pa.tile([P, T], f32, tag="att")
                            nc.tensor.matmul(sp[:], ktt[kt][pbase:pbase + DH, hko, :],
                                             QT[pbase:pbase + DH, hko, :],
                                             start=True, stop=True,
                                             tile_position=(pbase, 0))
                            ext = exp_.tile([P, T], bf16, tag="exs")
                            nc.scalar.activation(ext[:], sp[:], AF.Exp, scale=sc_att[:])
                            nc.vector.tensor_tensor(ext[:], ext[:], maskS[kt][:], ALU.mult)
                            vsl = (vt1[kt][:, h, :] if h < H // 2
                                   else vt2[kt][:, h - H // 2, :])
                            nc.tensor.matmul(avp[hi][:], vsl, ext[:],
                                             start=(kt == 0), stop=(kt == NKT - 1))
                    # softmax denominators + normalization for this head pair
                    for hi in range(2):
                        ac = aca.tile([DH + 1, T], bf16, tag=f"ac{hi}", name=f"ac{hi}")
                        nc.vector.tensor_copy(ac[:], avp[hi][:])
                        invd = evn.tile([DH + 1, T], f32, tag="invd")
                        nc.vector.reciprocal(invd[DH:DH + 1, :], avp[hi][DH:DH + 1, :])
                        invdr = evn.tile([DH + 1, T], bf16, tag="invdr")
                        nc.vector.tensor_copy(invdr[DH:DH + 1, :], invd[DH:DH + 1, :])
                        bcp = pa.tile([DH, T], f32, tag="att", name="bcp")
                        nc.tensor.matmul(bcp[:], ones[DH:DH + 1, 0:DH],
                                         invdr[DH:DH + 1, :], start=True, stop=True)
                        invb = eva.tile([DH, T], bf16, tag="invb")
                        nc.vector.tensor_copy(invb[:], bcp[:])
                        pbase = DH * hi
                        nc.vector.tensor_tensor(aoT[pbase:pbase + DH, hko, :],
                                                ac[0:DH, :], invb[:], ALU.mult)

                # ---- O projection + residual ----
                def o_consume(m, pt, bcol):
                    ot = evf.tile([P, T], f32, tag="evf")
                    if bcol is not None:
                        nc.scalar.activation(ot[:], pt[:], AF.Identity, bias=bcol[:])
                    else:
                        nc.vector.tensor_copy(ot[:], pt[:])
                    nc.vector.tensor_tensor(xT[:, m, :], xT[:, m, :], ot[:], ALU.add)
                proj_T(wo, li, aoT, bias_t[:, 16:24] if has_bias else None, o_consume)

                # ---- FFN ----
                normT = rmsnorm(f"norm2_{li}")
                hts = []
                wp3 = wproj[li].rearrange("(ko p) m -> p ko m", p=P)
                wu3 = wup[li].rearrange("(ko p) m -> p ko m", p=P)
                for m in range(FKO):
                    wtp = wp.tile([P, KO, P], bf16, tag="wlhs")
                    nc.sync.dma_start(wtp[:], wp3[:, :, m * P:(m + 1) * P])
                    ptp = ps.tile([P, T], f32, tag="mm512")
                    for ko in range(KO):
                        nc.tensor.matmul(ptp[:], wtp[:, ko], normT[:, ko], start=(ko == 0),
                                         stop=(ko == KO - 1))
                    wtu = wp.tile([P, KO, P], bf16, tag="wlhs")
                    nc.sync.dma_start(wtu[:], wu3[:, :, m * P:(m + 1) * P])
                    ptu = ps.tile([P, T], f32, tag="mm512")
                    for ko in range(KO):
                        nc.tensor.matmul(ptu[:], wtu[:, ko], normT[:, ko], start=(ko == 0),
                                         stop=(ko == KO - 1))
                    usb = evn.tile([P, T], f32, tag="uev")
                    if has_bias:
                        nc.scalar.activation(usb[:], ptu[:], AF.Identity,
                                             bias=bias_t[:, 40 + m:41 + m])
                        psb = evn.tile([P, T], f32, tag="pev")
                        nc.scalar.activation(psb[:], ptp[:], AF.Identity,
                                             bias=bias_t[:, 24 + m:25 + m])
                        gsb = evn.tile([P, T], bf16, tag="gev")
                        nc.vector.tensor_tensor(gsb[:], psb[:], usb[:], ALU.mult)
                    else:
                        nc.scalar.activation(usb[:], ptu[:], AF.Copy)
                        gsb = evn.tile([P, T], bf16, tag="gev")
                        nc.vector.tensor_tensor(gsb[:], ptp[:], usb[:], ALU.mult)
                    sgt = evn.tile([P, T], bf16, tag="sgt")
                    nc.scalar.activation(sgt[:], gsb[:], AF.Sigmoid)
                    ht = acp.tile([P, T], bf16, tag=f"acc{m}", name=f"ht{m}")
                    nc.vector.tensor_tensor(ht[:], gsb[:], sgt[:], ALU.mult)
                    hts.append(ht)
                wd3 = wdown[li].rearrange("(ko p) m -> p ko m", p=P)
                for m in range(KO):
                    wt = wdp.tile([P, FKO, P], bf16, tag="wdn")
                    nc.sync.dma_start(wt[:], wd3[:, :, m * P:(m + 1) * P])
                    pt = ps.tile([P, T], f32, tag="mm512")
                    for ko in range(FKO):
                        nc.tensor.matmul(pt[:], wt[:, ko], hts[ko][:], start=(ko == 0),
                                         stop=(ko == FKO - 1))
                    dt_ = evf.tile([P, T], f32, tag="evf")
                    if has_bias:
                        nc.scalar.activation(dt_[:], pt[:], AF.Identity,
                                             bias=bias_t[:, 56 + m:57 + m])
                    else:
                        nc.vector.tensor_copy(dt_[:], pt[:])
                    nc.vector.tensor_tensor(xT[:, m, :], xT[:, m, :], dt_[:], ALU.add)

        if want_xout:
            nc.sync.dma_start(xout[:], xT[:])

        if do_logits:
            xb = pb.tile([P, KO, T], bf16, tag="QT")
            nc.vector.tensor_copy(xb[:], xT[:])
            wl3 = wlog.rearrange("(ko p) m -> p ko m", p=P)
            with tc.tile_pool(name="wlp", bufs=3) as wlp, \
                 tc.tile_pool(name="lev", bufs=3) as lev:
                for nt in range(NVT):
                    wt = wlp.tile([P, KO, VN], bf16, tag="wlog")
                    nc.sync.dma_start(wt[:], wl3[:, :, nt * VN:(nt + 1) * VN])
                    blt = None
                    if blogmat is not None:
                        blt = wlp.tile([P, VN], bf16, tag="wlogb")
                        nc.sync.dma_start(blt[:], blogmat[:, nt * VN:(nt + 1) * VN])
                    for mt in range(T // P):
                        pt = ps.tile([P, VN], f32, tag="mm512")
                        for ko in range(KO):
                            last = (ko == KO - 1) and blogmat is None
                            nc.tensor.matmul(pt[:], xb[:, ko, mt * P:(mt + 1) * P],
                                             wt[:, ko], start=(ko == 0), stop=last)
                        if blt is not None:
                            nc.tensor.matmul(pt[:], e0[:, :P], blt[:],
                                             start=False, stop=True)
                        lo = lev.tile([P, VN], f32, tag="evl")
                        nc.vector.tensor_copy(lo[:], pt[:])
                        nc.sync.dma_start(
                            logits[mt * P:(mt + 1) * P, nt * VN:(nt + 1) * VN], lo[:])

    nc.compile()
    return nc


def host_inputs(inp, n_layers=4, do_logits=True):
    """Build per-core in_maps from the full model inputs dict (numpy)."""
    import math
    import ml_dtypes
    bf = ml_dtypes.bfloat16
    g = {k: np.asarray(v) for k, v in inp.items()}
    ids = g["input_ids"].astype(np.int64)
    embed = g["embed"].astype(np.float32)
    pos = np.arange(L, dtype=np.float32)[:, None]
    div = np.exp(np.arange(0, D, 2, dtype=np.float32) * (-math.log(10000.0) / D))
    ang = pos * div
    pe = np.zeros((L, D), dtype=np.float32)
    pe[:, 0::2] = np.sin(ang)
    pe[:, 1::2] = np.cos(ang)

    gam = g["gammas"].astype(np.float32)
    wq_ = np.ascontiguousarray(gam[0:2 * n_layers:2, :, None] * g["Wq"][:n_layers]).astype(bf)
    wk_ = np.ascontiguousarray(gam[0:2 * n_layers:2, :, None] * g["Wk"][:n_layers]).astype(bf)
    wv_ = np.ascontiguousarray(gam[0:2 * n_layers:2, :, None] * g["Wv"][:n_layers]).astype(bf)
    wo_ = np.ascontiguousarray(g["Wo"][:n_layers]).astype(bf)
    wp_ = np.ascontiguousarray(gam[1:2 * n_layers:2, :, None] * g["Wproj"][:n_layers]).astype(bf)
    wu_ = np.ascontiguousarray(gam[1:2 * n_layers:2, :, None] * g["Wup"][:n_layers]).astype(bf)
    wd_ = np.ascontiguousarray(g["Wdown"][:n_layers]).astype(bf)

    has_bias = bool(
        np.any(g["bq"][:n_layers]) or np.any(g["bk"][:n_layers])
        or np.any(g["bo"][:n_layers]) or np.any(g["bproj"][:n_layers])
        or np.any(g["bup"][:n_layers]) or np.any(g["bdown"][:n_layers]))
    nonzero_bv = bool(np.any(g["bv"][:n_layers] != 0))
    nonzero_blog = do_logits and bool(np.any(g["blogits"] != 0))

    ball = None
    if has_bias:
        ball = np.zeros((n_layers, P, 64), np.float32)
        for i in range(n_layers):
            ball[i, :, 0:8] = g["bq"][i].reshape(8, P).T
            ball[i, :, 8:16] = g["bk"][i].reshape(8, P).T
            ball[i, :, 16:24] = g["bo"][i].reshape(8, P).T
            ball[i, :, 24:40] = g["bproj"][i].reshape(16, P).T
            ball[i, :, 40:56] = g["bup"][i].reshape(16, P).T
            ball[i, :, 56:64] = g["bdown"][i].reshape(8, P).T
    bvmat = None
    if nonzero_bv:
        bvmat = np.zeros((n_layers, P, D), np.float32)
        bvmat[:, 0, :] = g["bv"][:n_layers]
        bvmat = bvmat.astype(bf)
    e0 = np.zeros((P, P), np.float32)
    e0[0, :] = 1.0

    consts = np.zeros((P, 2 * P), np.float32)
    consts[:, 0:P] = 1.0
    consts[0, P:P + DH] = 1.0        # bpat row 0 -> out partitions 0:64
    consts[1, P + DH:P + P] = 1.0    # bpat row 1 -> out partitions 64:128
    consts = consts.astype(bf)

    wlog_bf = None
    if do_logits:
        wlog_bf = np.ascontiguousarray(g["Wlogits"]).astype(bf)
        blogmat = None
        if nonzero_blog:
            blogmat = np.zeros((P, V), np.float32)
            blogmat[0, :] = g["blogits"]
            blogmat = blogmat.astype(bf)

    in_maps = []
    for c in range(NCORE):
        b, q = c // 4, c % 4
        tok = ids[b, q * T:(q + 1) * T]
        x0 = embed[tok] + pe[q * T:(q + 1) * T, :]
        i_idx = np.arange(P)[:, None]
        j_idx = np.arange(T)[None, :]
        maskbig = np.zeros((NKT, P, T), np.float32)
        for kt in range(NKT):
            maskbig[kt] = ((kt * P + i_idx) <= (q * T + j_idx)).astype(np.float32)
        m = {
            "x0T": np.ascontiguousarray(x0.T, np.float32),
            "wq": wq_, "wk": wk_, "wv": wv_, "wo": wo_,
            "wproj": wp_, "wup": wu_, "wdown": wd_,
            "consts": consts, "maskbig": maskbig.astype(bf),
        }
        if has_bias:
            m["ball"] = ball
        if nonzero_bv:
            m["bvmat"] = bvmat
        if nonzero_bv or nonzero_blog:
            m["e0_d"] = e0.astype(bf)
        if do_logits:
            m["wlog"] = wlog_bf
            if nonzero_blog:
                m["blogmat"] = blogmat
        in_maps.append(m)
    return in_maps, has_bias, nonzero_bv, nonzero_blog


_CACHE = {}


def _get_nc(key):
    if key not in _CACHE:
        _CACHE[key] = build(n_layers=4, do_logits=True, has_bias=key[0],
                            nonzero_bv=key[1], nonzero_blog=key[2])
    return _CACHE[key]


def kernel(**inputs):
    """Full-model entry: takes setup_inputs() arrays, returns [B, L, V] float32 logits."""
    from concourse.bass_utils import run_bass_kernel_spmd
    in_maps, hb, nzbv, nzbl = host_inputs(inputs, n_layers=4, do_logits=True)
    nc = _get_nc((hb, nzbv, nzbl))
    res = run_bass_kernel_spmd(nc, in_maps, core_ids=list(range(NCORE)))
    out = np.empty((B, L, V), np.float32)
    for c in range(NCORE):
        b, q = c // 4, c % 4
        out[b, q * T:(q + 1) * T, :] = res.results[c]["logits"]
    return np.ascontiguousarray(out, dtype=np.float32)
